# revision 37
# baseline (speedup 1.0000x reference)
"""Causal self-attention (RoPE + RMS-norm QK, 16 heads) on 8 Trainium2 cores.

Sharding: core c = (b, g) with b = c // 4 (batch), g = c % 4 (head group of 4).
Each core computes q/k/v projections for its 4 heads from x[b], runs causal
attention, and the out-projection restricted to its head-group columns of
wproj; the host sums the 4 partial outputs per batch.

Schedule (v7):
- everything bf16 on the input side (same PE rate as fp32r, half the DMA and
  SBUF traffic); fp32 only inside PSUM accumulation; bf16 output.
- phase P: single pass over x; per t-chunk the Q/K/V projections run with
  each unit's epilogue software-pipelined behind the next unit's matmuls.
  rms-norm = DVE reciprocal_approx_fast + ACT Sqrt (2 act tables total).
- phase D is scheduled around the scalar engine's exp throughput (~0.84us
  per [128,512] tile vs ~0.44us of PE work per tile): head h's QK matmuls
  and exps are emitted a full head ahead of head h-1's PV matmuls, and the
  out-projection pieces of the previous tq-chunk are spread into the gaps,
  so the in-order PE queue always has ready work while exps stream.
- softmax denominator: e tiles pair-combined on the idle GpSimd, serial
  chain on DVE in bf16 (host-validated identical accuracy), seeded from the
  first off-diag pair; diag bands join last (they are masked on GpSimd and
  would otherwise stall the chain head). One bf16 all-ones matmul per (j,h)
  broadcasts the column sums in fp32. For j=0 (diag-only) the denominator
  accumulates directly in PSUM via 4 column-restricted ones-matmuls.
- diagonal-band QK/exp/PV restricted to their causal columns; masks applied
  in place on GpSimd.
- out-projection drains (PSUM -> bf16 SBUF) go to DVE while the exp stream
  is hot and to the then-idle ACT at chunk-boundary flushes.

Per-core layout ("transposed-S"): projections produce Q^T/K^T with head-dim
on partitions, V in natural [t, d] layout. Scores are computed transposed
(S^T[tk, tq]) so softmax needs no transposes or max-subtraction (logits are
bounded by sqrt(D) after RMS-norm).
"""

import numpy as np
import ml_dtypes

import concourse.bass as bass
import concourse.mybir as mybir
import concourse.tile as tile
from concourse import bacc
from concourse.bass_utils import run_bass_kernel_spmd


def _ensure_ntff_hook():
    """If the environment requests NTFF tracing (BASS_TRACE) but the image's
    antenv lacks axon_hooks, install the same ctypes-based hook trn_boot
    would register. No-op when the real module exists."""
    import sys, types, contextlib
    try:
        from antenv.axon_hooks import get_axon_ntff_profile_hook  # noqa: F401
        return
    except ImportError:
        pass
    hook = None
    try:
        import ctypes
        lib = ctypes.CDLL("/opt/axon/libaxon_pjrt.so")
        if hasattr(lib, "axon_start_nrt_profile"):
            lib.axon_start_nrt_profile.argtypes = [
                ctypes.POINTER(ctypes.c_int64), ctypes.c_size_t]
            lib.axon_start_nrt_profile.restype = ctypes.c_int64
            lib.axon_stop_nrt_profile.argtypes = [ctypes.c_char_p]
            lib.axon_stop_nrt_profile.restype = ctypes.c_int64

            @contextlib.contextmanager
            def _hook(output_dir, device_ids):
                import jax
                jax.devices()
                if device_ids:
                    ids = (ctypes.c_int64 * len(device_ids))(*device_ids)
                    rc = lib.axon_start_nrt_profile(ids, len(device_ids))
                else:
                    rc = lib.axon_start_nrt_profile(None, 0)
                if rc != 0:
                    raise RuntimeError(f"axon_start_nrt_profile rc={rc}")
                try:
                    yield
                finally:
                    lib.axon_stop_nrt_profile(str(output_dir).encode())

            hook = _hook
    except OSError:
        pass
    import antenv
    mod = types.ModuleType("antenv.axon_hooks")
    mod.get_axon_ntff_profile_hook = lambda: hook
    mod.set_axon_ntff_profile_hook = lambda h: None
    sys.modules["antenv.axon_hooks"] = mod
    antenv.axon_hooks = mod
    # in this degraded environment there is no artifact store either
    from concourse import bass_utils
    bass_utils.upload_artifacts = lambda tmpdir: "local://" + tmpdir

P = 128          # partitions / head dim
T = 2048         # sequence length
C = 2048         # model dim
HL = 4           # heads per core
DL = HL * P      # local projection width (512)
NCO = C // P     # c-chunks (16)
XCH = 512        # x t-chunk width for projections
NXCH = T // XCH  # 4
QCH = 512        # tq-chunk width for attention
NQCH = T // QCH  # 4
NSTR = QCH // P  # diagonal-band tiles per chunk (4)
NTT = T // P     # t-tiles (16)

F32 = mybir.dt.float32
BF16 = mybir.dt.bfloat16
MUL = mybir.AluOpType.mult
SUB = mybir.AluOpType.subtract
ADD = mybir.AluOpType.add
SQRT = mybir.ActivationFunctionType.Sqrt
EXP = mybir.ActivationFunctionType.Exp


def build_program():
    nc = bacc.Bacc("TRN2", target_bir_lowering=False, debug=False, num_devices=8)

    xT = nc.dram_tensor("xT", [C, T], BF16, kind="ExternalInput")
    wqT = nc.dram_tensor("wqT", [C, DL], BF16, kind="ExternalInput")
    wkT = nc.dram_tensor("wkT", [C, DL], BF16, kind="ExternalInput")
    wvT = nc.dram_tensor("wvT", [C, DL], BF16, kind="ExternalInput")
    wpT = nc.dram_tensor("wpT", [DL, C], BF16, kind="ExternalInput")
    csA_d = nc.dram_tensor("csA", [P, T], BF16, kind="ExternalInput")   # cos|cos
    csB_d = nc.dram_tensor("csB", [P, T], BF16, kind="ExternalInput")   # sin|-sin
    tri_d = nc.dram_tensor("tri", [P, NSTR, QCH], BF16, kind="ExternalInput")
    ones_d = nc.dram_tensor("ones", [P, P], BF16, kind="ExternalInput")
    out_p = nc.dram_tensor("out_p", [T, C], BF16, kind="ExternalOutput")

    xT_r = xT.ap().rearrange("(co p) t -> p co t", p=P)

    with tile.TileContext(nc) as tc:
        with tc.tile_pool(name="base", bufs=1) as base:
            QT_sb = base.tile([P, HL, T], BF16, tag="QT")   # [d, h, tq]
            KT_sb = base.tile([P, HL, T], BF16, tag="KT")   # [d, h, tk]
            V_sb = base.tile([P, NTT, DL], BF16, tag="V")   # [t_sub, t_tile, d]
            ones_sb = base.tile([P, P], BF16, tag="ones")
            csA_sb = base.tile([P, T], BF16, tag="csA")
            csB_sb = base.tile([P, T], BF16, tag="csB")
            wp_sb = base.tile([P, HL, C], BF16, tag="wp")
            tri4_sb = base.tile([P, NSTR, QCH], BF16, tag="tri4")

            # ---- phase P: Q/K/V projections in one pass over x --------
            with (
                tc.tile_pool(name="pw", bufs=1) as pw,
                tc.tile_pool(name="px", bufs=2) as px,
                tc.tile_pool(name="pe1", bufs=2) as pe1,
                tc.tile_pool(name="pe2", bufs=2) as pe2,
                tc.tile_pool(name="ps_acc", bufs=3, space="PSUM") as ps_acc,
                tc.tile_pool(name="ps_ssq", bufs=2, space="PSUM") as ps_ssq,
            ):
                wq_sb = pw.tile([P, NCO, DL], BF16, tag="wq")
                wk_sb = pw.tile([P, NCO, DL], BF16, tag="wk")
                wv_sb = pw.tile([P, NCO, DL], BF16, tag="wv")
                wq_r = wqT.ap().rearrange("(co p) d -> p co d", p=P)
                wk_r = wkT.ap().rearrange("(co p) d -> p co d", p=P)
                wv_r = wvT.ap().rearrange("(co p) d -> p co d", p=P)

                def project_qk(x_sb, w_sb, h):
                    psq = ps_acc.tile([P, XCH], F32, tag="acc")
                    for c in range(NCO):
                        nc.tensor.matmul(
                            psq[:],
                            w_sb[:, c, h * P : (h + 1) * P],
                            x_sb[:, c, :],
                            start=(c == 0),
                            stop=(c == NCO - 1),
                        )
                    return psq

                def project_v(x_sb, m):
                    psv = ps_acc.tile([P, DL], F32, tag="acc")
                    for c in range(NCO):
                        nc.tensor.matmul(
                            psv[:],
                            x_sb[:, c, m * P : (m + 1) * P],
                            wv_sb[:, c, :],
                            start=(c == 0),
                            stop=(c == NCO - 1),
                        )
                    return psv

                def epilogue_qk(cols, dst_sb, h, scale, psq):
                    # RoPE fully in bf16 SBUF. csA = cos|cos, csB = sin|-sin,
                    # so tmp = [-q2*sin | q1*sin] with base-aligned reads and
                    # the combine is one full-height subtract.
                    qc = pe1.tile([P, XCH], BF16, tag="qc")
                    nc.scalar.copy(qc[:], psq[:])
                    tmp = pe2.tile([P, XCH], BF16, tag="tmp")
                    lo, hi = slice(0, 64), slice(64, P)
                    nc.vector.tensor_tensor(tmp[lo, :], qc[hi, :], csB_sb[hi, cols], MUL)
                    nc.vector.tensor_tensor(tmp[hi, :], qc[lo, :], csB_sb[lo, cols], MUL)
                    qr = pe1.tile([P, XCH], BF16, tag="qr")
                    nc.vector.tensor_tensor(qr[:], qc[:], csA_sb[:, cols], MUL)
                    nc.vector.tensor_tensor(qr[:], qr[:], tmp[:], SUB)
                    # RMS: ssq broadcast over partitions via all-ones lhsT;
                    # rinv = sqrt(scale / ssq) via DVE recip + ACT sqrt
                    q2t = pe2.tile([P, XCH], BF16, tag="q2t")
                    nc.vector.tensor_tensor(q2t[:], qr[:], qr[:], MUL)
                    ssq = ps_ssq.tile([P, XCH], F32, tag="ssq")
                    nc.tensor.matmul(ssq[:], ones_sb[:], q2t[:], start=True, stop=True)
                    r1 = pe2.tile([P, XCH], F32, tag="r1")
                    nc.vector.reciprocal_approx_fast(r1[:], ssq[:])
                    rinv = pe2.tile([P, XCH], BF16, tag="rinv")
                    nc.scalar.activation(rinv[:], r1[:], SQRT, scale=scale)
                    nc.vector.tensor_tensor(dst_sb[:, h, cols], qr[:], rinv[:], MUL)

                for tcx in range(NXCH):
                    cols = slice(tcx * XCH, (tcx + 1) * XCH)
                    x_sb = px.tile([P, NCO, XCH], BF16, tag="x")
                    if tcx == 0:
                        # interleave first-chunk DMAs so the first matmuls
                        # (wq head 0, x chunk 0, low c) start early
                        nc.sync.dma_start(wq_sb[:, 0:2, :], wq_r[:, 0:2, :])
                        nc.sync.dma_start(x_sb[:, 0:2, :], xT_r[:, 0:2, cols])
                        nc.sync.dma_start(wq_sb[:, 2:4, :], wq_r[:, 2:4, :])
                        nc.sync.dma_start(x_sb[:, 2:4, :], xT_r[:, 2:4, cols])
                        nc.sync.dma_start(ones_sb[:], ones_d.ap())
                        nc.sync.dma_start(csA_sb[:], csA_d.ap())
                        nc.sync.dma_start(csB_sb[:], csB_d.ap())
                        nc.sync.dma_start(wq_sb[:, 4:8, :], wq_r[:, 4:8, :])
                        nc.sync.dma_start(x_sb[:, 4:8, :], xT_r[:, 4:8, cols])
                        nc.sync.dma_start(wq_sb[:, 8:, :], wq_r[:, 8:, :])
                        nc.sync.dma_start(x_sb[:, 8:, :], xT_r[:, 8:, cols])
                        nc.sync.dma_start(wk_sb[:], wk_r[:])
                        nc.sync.dma_start(wv_sb[:], wv_r[:])
                        nc.sync.dma_start(
                            wp_sb[:], wpT.ap().rearrange("(h p) j -> p h j", p=P)
                        )
                        nc.sync.dma_start(tri4_sb[:], tri_d.ap())
                    else:
                        nc.sync.dma_start(x_sb[:], xT_r[:, :, cols])

                    # software-pipeline: unit u's epilogue is interleaved
                    # behind unit u+1's projection matmuls
                    units = (
                        [("q", h) for h in range(HL)]
                        + [("k", h) for h in range(HL)]
                        + [("v", m) for m in range(XCH // P)]
                    )
                    def run_epilogue(pkind, pidx, pps):
                        if pkind == "q":
                            epilogue_qk(cols, QT_sb, pidx, 1.0, pps)
                        elif pkind == "k":
                            epilogue_qk(cols, KT_sb, pidx, float(P), pps)
                        else:
                            nc.scalar.copy(
                                V_sb[:, tcx * (XCH // P) + pidx, :], pps[:]
                            )

                    # two-unit lookahead: each epilogue's ACT/DVE chain gets
                    # two projection blocks of time before its ssq matmul
                    # appears in the PE queue
                    pend_epi = []
                    for kind, idx in units:
                        if kind == "q":
                            ps = project_qk(x_sb, wq_sb, idx)
                        elif kind == "k":
                            ps = project_qk(x_sb, wk_sb, idx)
                        else:
                            ps = project_v(x_sb, idx)
                        pend_epi.append((kind, idx, ps))
                        if len(pend_epi) > 2:
                            run_epilogue(*pend_epi.pop(0))
                    for item in pend_epi:
                        run_epilogue(*item)

            # ---- phase D: attention + out-projection, exp-paced -------
            with (
                tc.tile_pool(name="de", bufs=26) as de,
                tc.tile_pool(name="de4", bufs=3) as de4,
                tc.tile_pool(name="dsum", bufs=3) as dsum,
                tc.tile_pool(name="dm", bufs=2) as dm,
                tc.tile_pool(name="ps_st", bufs=3, space="PSUM") as ps_st,
                tc.tile_pool(name="ps_ot", bufs=2, space="PSUM") as ps_ot,
                tc.tile_pool(name="ps_den", bufs=1, space="PSUM") as ps_den,
                tc.tile_pool(name="ps_po", bufs=2, space="PSUM") as ps_po,
            ):

                def emit_qk(j, h, i, state):
                    """One QK tile + exp (+mask / esum bookkeeping)."""
                    e4, e_list, esum, pend, pairs = state
                    noff = j * NSTR
                    r = i - noff
                    if r >= 0:
                        cc = slice(r * P, QCH)
                        st = ps_st.tile([P, QCH], F32, tag="st")
                        nc.tensor.matmul(
                            st[:, cc],
                            KT_sb[:, h, i * P : (i + 1) * P],
                            QT_sb[:, h, j * QCH + r * P : (j + 1) * QCH],
                            start=True,
                            stop=True,
                        )
                        nc.scalar.activation(e4[:, r, cc], st[:, cc], EXP)
                        nc.gpsimd.tensor_tensor(
                            e4[:, r, cc], e4[:, r, cc], tri4_sb[:, r, cc], MUL
                        )
                        return
                    st = ps_st.tile([P, QCH], F32, tag="st")
                    nc.tensor.matmul(
                        st[:],
                        KT_sb[:, h, i * P : (i + 1) * P],
                        QT_sb[:, h, j * QCH : (j + 1) * QCH],
                        start=True,
                        stop=True,
                    )
                    e_sb = de.tile([P, QCH], BF16, tag="e")
                    nc.scalar.activation(e_sb[:], st[:], EXP)
                    e_list.append(e_sb)
                    # pairs combine on the idle GpSimd; the DVE chain is
                    # seeded from the first two pairs
                    pend.append(e_sb[:])
                    if len(pend) == 2:
                        pair = dsum.tile([P, QCH], BF16, tag="pair")
                        nc.gpsimd.tensor_tensor(pair[:], pend[0], pend[1], ADD)
                        pend.clear()
                        pairs.append(pair)
                        if len(pairs) == 2:
                            nc.vector.tensor_tensor(
                                esum[:], pairs[0][:], pairs[1][:], ADD
                            )
                        elif len(pairs) > 2:
                            nc.vector.tensor_tensor(esum[:], esum[:], pair[:], ADD)

                def qk_tail(j, state):
                    # diag bands join the chain last: their GpSimd masks have
                    # finished and the chain head never waited on them
                    e4, e_list, esum, pend, pairs = state
                    if j > 0:
                        for r in range(NSTR):
                            cc = slice(r * P, QCH)
                            nc.vector.tensor_tensor(
                                esum[:, cc], esum[:, cc], e4[:, r, cc], ADD
                            )

                def emit_pv(j, h, i, state, ot_ps):
                    e4, e_list, esum, pend, pairs = state
                    noff = j * NSTR
                    ntk = noff + NSTR
                    r = i - noff
                    if r >= 0:
                        cc = slice(r * P, QCH)
                        e_ap = e4[:, r, cc]
                        ocols = cc
                    else:
                        e_ap = e_list[i][:]
                        ocols = slice(0, QCH)
                    nc.tensor.matmul(
                        ot_ps[:, ocols],
                        V_sb[:, i, h * P : (h + 1) * P],
                        e_ap,
                        start=(i == 0),
                        stop=(i == ntk - 1),
                    )

                def pv_tail(j, h, state, ot_ps, ot_ch):
                    e4, e_list, esum, pend, pairs = state
                    den_ps = ps_den.tile([P, QCH], F32, tag="den")
                    if j == 0:
                        # diag-only chunk: accumulate the denominator
                        # directly in PSUM, no DVE chain to wait for
                        for r in range(NSTR):
                            cc = slice(r * P, QCH)
                            nc.tensor.matmul(
                                den_ps[:, cc],
                                ones_sb[:],
                                e4[:, r, cc],
                                start=(r == 0),
                                stop=(r == NSTR - 1),
                            )
                    else:
                        nc.tensor.matmul(
                            den_ps[:], ones_sb[:], esum[:], start=True, stop=True
                        )
                    recip = dm.tile([P, QCH], F32, tag="recip")
                    nc.vector.reciprocal_approx_fast(recip[:], den_ps[:])
                    nc.vector.tensor_tensor(ot_ch[:, h, :], ot_ps[:], recip[:], MUL)

                po_tog = [0]

                def emit_po(jp, ot_prev, piece, drain=None):
                    u, jc = piece
                    po = ps_po.tile([P, QCH], F32, tag="po")
                    for h in range(HL):
                        nc.tensor.matmul(
                            po[:],
                            ot_prev[:, h, u * P : (u + 1) * P],
                            wp_sb[:, h, jc * QCH : (jc + 1) * QCH],
                            start=(h == 0),
                            stop=(h == HL - 1),
                        )
                    osb = dm.tile([P, QCH], BF16, tag="osb")
                    # alternate drains between ACT and DVE so the two po
                    # PSUM banks are never both stuck behind one engine's
                    # queue (the exp stream has a full head-period of slack)
                    po_tog[0] ^= 1
                    if po_tog[0]:
                        nc.scalar.copy(osb[:], po[:])
                    else:
                        nc.vector.tensor_copy(osb[:], po[:])
                    nc.sync.dma_start(
                        out_p.ap()[
                            jp * QCH + u * P : jp * QCH + (u + 1) * P,
                            jc * QCH : (jc + 1) * QCH,
                        ],
                        osb[:],
                    )

                # Slot schedule per chunk: each slot leads with guaranteed-
                # ready work (prev head's PV, prev chunk's out-proj pieces)
                # and ends with the exp-paced QK, so the in-order PE queue
                # never idles on the scalar engine.
                all_pieces = [(u, jc) for u in range(QCH // P) for jc in range(NQCH)]
                prev = None  # (j, ot_ch) awaiting out-projection
                for j in range(NQCH):
                    ntk = (j + 1) * NSTR
                    ot_ch = dm.tile([P, HL, QCH], BF16, tag="otch")
                    nslots = (HL + 1) * ntk
                    pieces = list(all_pieces) if prev is not None else []
                    pcadence = nslots / 16.0
                    pacc = 0.0
                    slot = 0

                    def slot_po(drain="dve"):
                        nonlocal pacc, slot
                        slot += 1
                        pacc += 1.0
                        while pieces and pacc >= pcadence:
                            pacc -= pcadence
                            emit_po(prev[0], prev[1], pieces.pop(0), drain=drain)

                    states = {}
                    ots = {}
                    for h in range(HL):
                        e4_t = de4.tile([P, NSTR, QCH], BF16, tag="e4")
                        esum_t = dsum.tile([P, QCH], BF16, tag="esum")
                        states[h] = (e4_t, [], esum_t, [], [])
                        if h >= 1:
                            ot_t = ps_ot.tile([P, QCH], F32, tag="ot")
                            ots[h - 1] = ot_t
                        for i in range(ntk):
                            if h >= 1:
                                emit_pv(j, h - 1, i, states[h - 1], ots[h - 1])
                            slot_po()
                            emit_qk(j, h, i, states[h])
                        if h >= 1:
                            pv_tail(j, h - 1, states[h - 1], ots[h - 1], ot_ch)
                            del states[h - 1]
                        qk_tail(j, states[h])
                    ot_l = ps_ot.tile([P, QCH], F32, tag="ot")
                    ots[HL - 1] = ot_l
                    for i in range(ntk):
                        emit_pv(j, HL - 1, i, states[HL - 1], ots[HL - 1])
                        slot_po(drain="act")
                    pv_tail(j, HL - 1, states[HL - 1], ots[HL - 1], ot_ch)
                    while pieces:
                        emit_po(prev[0], prev[1], pieces.pop(0), drain="act")
                    prev = (j, ot_ch)
                for n, piece in enumerate(all_pieces):
                    emit_po(prev[0], prev[1], piece,
                            drain="act" if n % 2 else "dve")

    nc.compile()
    return nc


_NC = None


def _get_nc():
    global _NC
    if _NC is None:
        _NC = build_program()
    return _NC


def _host_inputs(x, cos, sin, wq, wk, wv, wproj):
    BF = ml_dtypes.bfloat16
    B = x.shape[0]
    cosT = np.ascontiguousarray(cos[0, :, 0, :].T).astype(np.float32)  # [64, T]
    sinT = np.ascontiguousarray(sin[0, :, 0, :].T).astype(np.float32)
    csA = np.concatenate([cosT, cosT], axis=0).astype(BF)
    csB = np.concatenate([sinT, -sinT], axis=0).astype(BF)
    # tri[p, r, f] = 1 iff causal (tk=128r+p <= tq=f) within a diagonal band
    rr, pp, ff = np.meshgrid(np.arange(NSTR), np.arange(P), np.arange(QCH), indexing="ij")
    tri = np.ascontiguousarray(
        (pp + 128 * rr <= ff).astype(np.float32).transpose(1, 0, 2)
    ).astype(BF)
    ones = np.ones((P, P), BF)

    xTs = [np.ascontiguousarray(x[b].T).astype(BF) for b in range(B)]
    in_maps = []
    for core in range(8):
        b, g = divmod(core, 4)
        sl = slice(g * DL, (g + 1) * DL)
        in_maps.append({
            "xT": xTs[b],
            "wqT": np.ascontiguousarray(wq[sl, :].T).astype(BF),
            "wkT": np.ascontiguousarray(wk[sl, :].T).astype(BF),
            "wvT": np.ascontiguousarray(wv[sl, :].T).astype(BF),
            "wpT": np.ascontiguousarray(wproj[:, sl].T).astype(BF),
            "csA": csA, "csB": csB, "tri": tri, "ones": ones,
        })
    return in_maps


def kernel(x, cos, sin, wq, wk, wv, wproj, _trace=False):
    _ensure_ntff_hook()
    nc = _get_nc()
    in_maps = _host_inputs(x, cos, sin, wq, wk, wv, wproj)
    res = run_bass_kernel_spmd(nc, in_maps, core_ids=list(range(8)), trace=_trace)
    parts = [res.results[c]["out_p"].astype(np.float32) for c in range(8)]
    out = np.stack([
        sum(parts[0:4]),
        sum(parts[4:8]),
    ]).astype(np.float32)
    kernel.last_exec_time_ns = res.exec_time_ns
    kernel.last_result = res
    return out
